# revision 21
# baseline (speedup 1.0000x reference)
"""Trainium2 Bass kernel for causal multi-head attention with QKV projections.

Problem: x [2, 4096, 1024], Wq/Wk/Wv [1024, 1024] (nn.Linear, y = x @ W.T),
16 heads x 64 dim, causal softmax attention, output [2, 4096, 1024] fp32.

Sharding: 8 cores; core c handles batch b = c // 4 and heads
[4*(c%4), 4*(c%4)+4)  (4 heads = 256 channels per core). No cross-core comm.

Per-core device pipeline (matmul streams in bf16, fp32 PSUM accumulate):
  - Attention runs as ONE continuous software pipeline over every
    (head-pair, 512-query group, 128-key tile): per key tile, two concurrent
    QK matmuls (head A at partitions 0-63, head B at 64-127 -> distinct PE
    row groups) write S^T[k, q] for both heads into one PSUM tile; one exp
    on ACT (PSUM -> SBUF bf16); causal zeroing of the diagonal-tile triangle
    via gpsimd affine_select; then per head O^T[128, 512] += V_ext^T E^T
    accumulates in PSUM.  Diagonal key tiles are trimmed: QK / exp / AV only
    touch query columns >= m*128 (m = key-tile index within the group), the
    affine_select covers just the [128,128] triangle block.
  - V tiles carry the two heads side by side ([keys, h0 dims | h1 dims]) so
    one 128x128 PE transpose produces both heads' V for a key tile.  The AV
    lhsT reads its head's 64 columns plus a shared 64-wide ones block via a
    two-block strided AP, giving a 128-column weight load (FWL-eligible);
    output rows 64.. hold the softmax denominators.
  - The QKV projections (Q^T/K^T/V^T = W^T-slice @ x^T, 8 contraction tiles,
    N=512 bf16) and the V^T -> V PE transposes are emitted as staged FILLER
    between attention instructions: gated so a group's inputs are always
    emitted first, rationed so the PE never idles long enough for the HAM
    clock gate to re-throttle (that costs 2x PE speed when it happens).
  - Finalize per (head, group): O^T -> SBUF bf16, PE-transpose 128-query
    blocks to [128, 128]; column 64 holds the denominators -> reciprocal +
    per-partition scale; one batched DMA per (head, group) to DRAM.
  - The 1/sqrt(D) score scale is folded into Wq on the host.
"""

import sys

import numpy as np

try:  # the axon site normally provides concourse; fall back to the repo copy
    import concourse  # noqa: F401
except ImportError:  # pragma: no cover
    sys.path.insert(0, "/opt/trn_rl_repo")

from contextlib import ExitStack

import ml_dtypes
import concourse.bass as bass  # noqa: F401  (AP helpers)
import concourse.tile as tile
from concourse import bacc, bass_utils, mybir
from concourse.masks import make_identity

FP = mybir.dt.float32
BF = mybir.dt.bfloat16
AF = mybir.ActivationFunctionType

B, T_FULL, C = 2, 4096, 1024
H, D = 16, 64
N_CORES = 8
HPC = 4            # heads per core
CPC = HPC * D      # channels per core (256)
QG = 512           # query-group width

_CACHE = {}


def _emit(ctx, tc, t):
    nc = tc.nc
    nkt = t // 128       # key tiles
    nqg = t // QG        # query groups
    ntc = t // 512       # projection t-chunks

    xT = nc.dram_tensor("xT", [C, t], BF, kind="ExternalInput").ap()
    wqT = nc.dram_tensor("wqT", [C, CPC], BF, kind="ExternalInput").ap()
    wkT = nc.dram_tensor("wkT", [C, CPC], BF, kind="ExternalInput").ap()
    wvT = nc.dram_tensor("wvT", [C, CPC], BF, kind="ExternalInput").ap()
    out = nc.dram_tensor("out", [t, CPC], FP, kind="ExternalOutput").ap()

    # ---------------- persistent SBUF ----------------
    big = ctx.enter_context(tc.tile_pool(name="big", bufs=1))
    ident_fp = big.tile([128, 128], FP, tag="ident_fp")
    make_identity(nc, ident_fp)
    ident = big.tile([128, 128], BF, tag="ident")
    nc.vector.tensor_copy(ident, ident_fp)

    # Q^T / K^T head-pair tiles: partition p -> head hp*2 + p//64, dim p%64
    qt = [big.tile([128, t], BF, tag=f"qt{hp}", name=f"qt{hp}") for hp in range(HPC // 2)]
    kt = [big.tile([128, t], BF, tag=f"kt{hp}", name=f"kt{hp}") for hp in range(HPC // 2)]
    # V (bf16): per (pair, key tile, head): 64 dims | 64-wide ones block,
    # so the AV lhsT is a contiguous FWL-eligible 128-column weight whose
    # output rows 64.. hold the softmax denominators.
    v_sb = big.tile([128, HPC // 2, nkt, 2, 128], BF, tag="v_sb")

    # w tiles are used by both the upfront (head-pair 0) projections and
    # the filler (head-pair 1) projections interleaved into attention
    xtp = ctx.enter_context(tc.tile_pool(name="xtp", bufs=4))
    xT_r = xT.rearrange("(k p) t -> p k t", p=128)
    pn = 512

    # Interleave the first x chunk's DMAs with the weight DMAs k-first, so
    # the first projection matmul (needs x[k=0] + wq[k=0]) is fed after ~2
    # transfers instead of waiting out the whole DMA queue.  The second x
    # chunk is also prefetched up front so the stage-1 filler never stalls
    # the PE on its x transfer.
    w_sb = {}
    w_src = {}
    for name, dram in (("wq", wqT), ("wk", wkT), ("wv", wvT)):
        w_sb[name] = big.tile([128, 8, CPC], BF, tag=name, name=f"w_{name}")
        w_src[name] = dram.rearrange("(k p) m -> p k m", p=128)
    x_first = xtp.tile([128, 8, pn], BF, tag="x", name="x_first")
    for k in range(8):
        nc.sync.dma_start(x_first[:, k, :], xT_r[:, k, 0:pn])
        for name in ("wq", "wk", "wv"):
            nc.sync.dma_start(w_sb[name][:, k, :], w_src[name][:, k, :])

    # x chunks are prefetched two stages ahead of the filler cursor so a
    # stage's projections never wait on its own x transfer.
    x_tiles = {0: x_first}
    stage_keys = [(hp, ch) for hp in range(HPC // 2) for ch in range(ntc)]

    def prefetch_x(s):
        if s >= len(stage_keys) or s in x_tiles:
            return
        hp, ch = stage_keys[s]
        tsl = slice(ch * pn, (ch + 1) * pn)
        x_tile = xtp.tile([128, 8, pn], BF, tag="x", name=f"x{hp}_{ch}")
        for k in range(8):
            nc.sync.dma_start(x_tile[:, k, :], xT_r[:, k, tsl])
        x_tiles[s] = x_tile

    prefetch_x(1)
    prefetch_x(2)

    spsum = ctx.enter_context(tc.tile_pool(name="spsum", bufs=2, space="PSUM"))
    opsum = ctx.enter_context(tc.tile_pool(name="opsum", bufs=2, space="PSUM"))
    fillp = ctx.enter_context(tc.tile_pool(name="fillp", bufs=2, space="PSUM"))
    ep = ctx.enter_context(tc.tile_pool(name="ep", bufs=8))
    fin = ctx.enter_context(tc.tile_pool(name="fin", bufs=4))
    fw = ctx.enter_context(tc.tile_pool(name="fw", bufs=1))

    # ones block once (cols 64..127 of every per-head v_sb row)
    nc.vector.memset(v_sb[:, :, :, :, D:128], 1.0)
    vt = [fw.tile([128, t], BF, tag=f"vt{hp}", name=f"vt{hp}") for hp in range(HPC // 2)]

    # ---- staged filler: projections + V transposes, one stage per (pair,
    # chunk).  Stage s = hp*ntc + ch emits chunk ch of pair hp's Q^T/K^T/V^T
    # plus the V transposes for that chunk's four key tiles; attention gates
    # on stages so inputs are always emitted before they are read.
    def make_stage(hp, ch):
        def gen():
            s = hp * ntc + ch
            tsl = slice(ch * pn, (ch + 1) * pn)
            x_tile = x_tiles.pop(s)
            prefetch_x(s + 2)
            yield
            dests = [(w_sb["wq"], qt[hp]), (w_sb["wk"], kt[hp]), (w_sb["wv"], vt[hp])]
            for di, (w_tile, dst) in enumerate(dests):
                pp = fillp.tile([128, pn], FP, tag="fill", name=f"pp{hp}_{ch}_{di}")
                for k in range(8):
                    nc.tensor.matmul(
                        pp,
                        lhsT=w_tile[:, k, hp * 128 : (hp + 1) * 128],
                        rhs=x_tile[:, k, :],
                        start=(k == 0),
                        stop=(k == 7),
                    )
                    yield
                nc.vector.tensor_copy(dst[:, tsl], pp)
                yield
            # both heads' V for key tile j in one 128x128 transpose
            for j in range(4 * ch, min(4 * ch + 4, nkt)):
                pt = fillp.tile([128, 128], BF, tag="fill", name=f"pt{hp}_{j}")
                nc.tensor.transpose(
                    pt,
                    vt[hp][:, j * 128 : (j + 1) * 128],
                    ident,
                )
                nc.vector.tensor_copy(
                    v_sb[:, hp, j, :, 0:D],
                    pt.rearrange("p (a b) -> p a b", b=D),
                )
                yield
        return gen()

    stages = [make_stage(hp, ch) for hp in range(HPC // 2) for ch in range(ntc)]
    cursor = {"i": 0}

    def feed(n):
        done = 0
        while done < n and cursor["i"] < len(stages):
            if next(stages[cursor["i"]], StopIteration) is StopIteration:
                cursor["i"] += 1
            else:
                done += 1

    def gate(s_idx):
        while cursor["i"] <= s_idx:
            if next(stages[cursor["i"]], StopIteration) is StopIteration:
                cursor["i"] += 1

    # ---------------- attention: one continuous pipeline ----------------
    def make_group(hp, g):
        jmax = (g + 1) * (QG // 128) - 1
        st = {"ots": None}

        def emit_qk(j):
            m = j - (g * QG) // 128  # key-tile index within the group (diag if >= 0)
            q0 = max(m, 0) * 128     # first query column that can attend this tile
            qsl = slice(g * QG + q0, (g + 1) * QG)
            sp = spsum.tile([128, 2, QG], FP, tag="sp", name=f"sp{hp}_{g}_{j}")
            e = ep.tile([128, 2, QG], BF, tag="e", name=f"e{hp}_{g}_{j}")
            for hh in (0, 1):
                po = 64 * hh
                nc.tensor.matmul(
                    sp[:, hh, q0:QG],
                    lhsT=kt[hp][po : po + 64, j * 128 : (j + 1) * 128],
                    rhs=qt[hp][po : po + 64, qsl],
                    start=True,
                    stop=True,
                )
            nc.scalar.activation(
                out=e[:, :, q0:QG],
                in_=sp[:, :, q0:QG],
                func=AF.Exp,
            )
            if m >= 0:  # diagonal tile: causal zeroing of the triangle block
                for hh in (0, 1):
                    blk = e[:, hh, q0 : q0 + 128]
                    nc.gpsimd.affine_select(
                        out=blk,
                        in_=blk,
                        compare_op=mybir.AluOpType.is_ge,
                        fill=0.0,
                        base=0,
                        pattern=[[1, 128]],
                        channel_multiplier=-1,
                    )
            return e, q0

        def emit_av(j, e, q0):
            if st["ots"] is None:
                st["ots"] = [
                    opsum.tile([128, QG], FP, tag="ot", name=f"ot{hp}_{g}_{hh}")
                    for hh in (0, 1)
                ]
            for hh in (0, 1):
                nc.tensor.matmul(
                    st["ots"][hh][:, q0:QG],
                    lhsT=v_sb[:, hp, j, hh, :],
                    rhs=e[:, hh, q0:QG],
                    start=(j == 0),
                    stop=(j == jmax),
                )
            if j == jmax:
                finalize()

        def finalize():
            qsl = slice(g * QG, (g + 1) * QG)
            for hh, h in ((0, 2 * hp), (1, 2 * hp + 1)):
                osb = fin.tile([128, QG], BF, tag="osb")
                nc.vector.tensor_copy(osb, st["ots"][hh])
                o_sb = fin.tile([128, QG // 128, D], FP, tag="o_sb")
                for qb in range(QG // 128):
                    ft = opsum.tile([128, 128], BF, tag="ot", name=f"ft{qb}")
                    nc.tensor.transpose(
                        ft,
                        osb[:, qb * 128 : (qb + 1) * 128],
                        ident,
                    )
                    recip = fin.tile([128, 1], FP, tag="recip")
                    nc.vector.reciprocal(recip, ft[:, D : D + 1])
                    nc.vector.tensor_scalar_mul(o_sb[:, qb, :], ft[:, 0:D], recip)
                nc.sync.dma_start(
                    out[qsl, h * D : (h + 1) * D].rearrange(
                        "(qb p) d -> p qb d", p=128
                    ),
                    o_sb,
                )

        return emit_qk, emit_av, jmax

    # Key tiles are emitted in adjacent pairs (j, j+1) so the two QK
    # head-pair episodes share one entry/exit into the split-row PE regime
    # (the LDW-vs-stream exposure is paid per episode boundary).
    # Pair 1 runs its groups rotated (1..7 then 0) so the kernel's serial
    # tail is the SMALLEST group's finalize instead of the largest one.
    pending = []
    for hp in range(HPC // 2):
        groups = list(range(nqg))
        if hp == HPC // 2 - 1 and nqg > 1:
            groups = groups[1:] + groups[:1]
        for g in groups:
            gate(hp * ntc + g)
            emit_qk, emit_av, jmax = make_group(hp, g)
            for j in range(0, jmax + 1, 2):
                e0, q00 = emit_qk(j)
                e1, q01 = emit_qk(j + 1)
                pending.append((emit_av, j, e0, q00))
                pending.append((emit_av, j + 1, e1, q01))
                feed(2)
                while len(pending) > 4:
                    av, jj, ee, qq0 = pending.pop(0)
                    av(jj, ee, qq0)
                    feed(1)
    for av, jj, ee, qq0 in pending:
        av(jj, ee, qq0)
    feed(10 ** 9)


def build_program(t=T_FULL):
    if t in _CACHE:
        return _CACHE[t]
    nc = bacc.Bacc("TRN2", target_bir_lowering=False, debug=False)
    with tile.TileContext(nc) as tc:
        with ExitStack() as ctx:
            _emit(ctx, tc, t)
    nc.compile()
    _CACHE[t] = nc
    return nc


def make_in_maps(x, Wq, Wk, Wv):
    """Host-side shard: returns the 8 per-core input maps."""
    x = np.asarray(x, dtype=np.float32)
    Wq = np.asarray(Wq, dtype=np.float32)
    Wk = np.asarray(Wk, dtype=np.float32)
    Wv = np.asarray(Wv, dtype=np.float32)
    scale = np.float32(D ** -0.5)
    bf = ml_dtypes.bfloat16
    xT = np.ascontiguousarray(x.transpose(0, 2, 1)).astype(bf)  # [B, C, T]
    in_maps = []
    for core in range(N_CORES):
        b, hg = divmod(core, N_CORES // B)
        sl = slice(hg * CPC, (hg + 1) * CPC)
        in_maps.append(
            {
                "xT": xT[b],
                "wqT": (np.ascontiguousarray(Wq[sl].T) * scale).astype(bf),
                "wkT": np.ascontiguousarray(Wk[sl].T).astype(bf),
                "wvT": np.ascontiguousarray(Wv[sl].T).astype(bf),
            }
        )
    return in_maps


LAST_RESULTS = None


def kernel(x, Wq, Wk, Wv, _trace=False):
    global LAST_RESULTS
    in_maps = make_in_maps(x, Wq, Wk, Wv)
    nc = build_program(T_FULL)
    res = bass_utils.run_bass_kernel_spmd(
        nc, in_maps, core_ids=list(range(N_CORES)), trace=_trace
    )
    LAST_RESULTS = res
    full = np.empty((B, T_FULL, C), np.float32)
    for core in range(N_CORES):
        b, hg = divmod(core, N_CORES // B)
        full[b, :, hg * CPC : (hg + 1) * CPC] = res.results[core]["out"]
    return full
